# revision 2
# baseline (speedup 1.0000x reference)
"""Block-diagonal grouped GEMM (GroupLinear) on 8 TRN2 NeuronCores.

Problem: x [8, 2048, 4096] f32, W [4096, 4096] f32 where only the 64
diagonal 64x64 blocks of W are used:
    y[b,s, g*64+o] = sum_i x[b,s, g*64+i] * W[g*64+o, g*64+i]

Strategy:
  - Data-parallel over batch: core b handles x[b] (2048 tokens).
  - All device I/O in bf16 (the rel-err budget is 2e-2; bf16 costs
    ~4e-3), halving HBM traffic vs f32: 16MB x-load + 16MB y-store +
    1MB weights per core against a ~358 GB/s per-core HBM limit.
  - Host feeds each core xT = x[b].T  [4096, 2048] (channel-major) so
    the PE contraction dim (input channel) is already on partitions ->
    no on-device transposes at all.
  - Two 64-ch groups pack into one 128-wide block-diagonal weight strip
    B_c [128i, 128o]; 32 strips total, resident in SBUF.
  - Per strip: load xT strip [128, 2048] bf16 (one contiguous 512KB
    DMA), 4 matmuls of [K=128] x [128, 512] -> PSUM f32, copy+cast
    PSUM->SBUF bf16 (alternating Vector/Scalar engines), DMA yT strip
    out in bf16.
  - Host transposes yT back and upcasts to f32. All device traffic is
    perfectly contiguous.
"""

import numpy as np
from ml_dtypes import bfloat16

import concourse.bacc as bacc
import concourse.mybir as mybir
from concourse.tile import TileContext
from concourse.bass_utils import run_bass_kernel_spmd

B, S, C = 8, 2048, 4096
G, GS = 64, 64            # groups, group size (=in_scale=out_scale)
NSTRIP = C // 128         # 32 strips of 128 channels (2 groups each)
TOK = 512                 # PSUM bank limit: 512 f32 per partition
FP32 = mybir.dt.float32
BF16 = mybir.dt.bfloat16


def _build_program():
    nc = bacc.Bacc()
    xt = nc.declare_dram_parameter("xt", [C, S], BF16, isOutput=False)
    wb = nc.declare_dram_parameter("wb", [128, NSTRIP * 128], BF16, isOutput=False)
    yt = nc.declare_dram_parameter("yt", [C, S], BF16, isOutput=True)

    with TileContext(nc) as tc:
        with (
            tc.tile_pool(name="wpool", bufs=1) as wpool,
            tc.tile_pool(name="xpool", bufs=8) as xpool,
            tc.tile_pool(name="opool", bufs=8) as opool,
            tc.tile_pool(name="ppool", bufs=8, space="PSUM") as ppool,
        ):
            # Weight load split across both HWDGE rings so all strips'
            # weights are resident ~2x sooner.
            w_sb = wpool.tile([128, NSTRIP * 128], BF16)
            half = NSTRIP * 128 // 2
            nc.sync.dma_start(out=w_sb[:, :half], in_=wb[:, :half])
            nc.scalar.dma_start(out=w_sb[:, half:], in_=wb[:, half:])
            for c in range(NSTRIP):
                # Loads issue on the Sync HWDGE ring; stores on the Scalar
                # HWDGE ring. A single ring stalls loads behind stores that
                # wait on compute semaphores (FIFO per issuing engine).
                x_t = xpool.tile([128, S], BF16)
                if c == 0:
                    # Scalar ring is idle during pipeline fill: split the
                    # first strip across both rings to start compute sooner.
                    nc.sync.dma_start(
                        out=x_t[:, :S // 2], in_=xt[:128, :S // 2]
                    )
                    nc.scalar.dma_start(
                        out=x_t[:, S // 2:], in_=xt[:128, S // 2:]
                    )
                else:
                    nc.sync.dma_start(out=x_t[:], in_=xt[c * 128:(c + 1) * 128, :])
                last = c == NSTRIP - 1
                for h in range(2):
                    o_t = opool.tile([128, S // 2], BF16)
                    for q in range(2):
                        tb = 2 * h + q
                        ps = ppool.tile([128, TOK], FP32)
                        nc.tensor.matmul(
                            out=ps[:],
                            lhsT=w_sb[:, c * 128:(c + 1) * 128],
                            rhs=x_t[:, tb * TOK:(tb + 1) * TOK],
                            start=True,
                            stop=True,
                        )
                        if (c * (S // TOK) + tb) % 2 == 0:
                            nc.vector.tensor_copy(
                                out=o_t[:, q * TOK:(q + 1) * TOK], in_=ps[:]
                            )
                        else:
                            nc.scalar.copy(
                                out=o_t[:, q * TOK:(q + 1) * TOK], in_=ps[:]
                            )
                        if last:
                            # Sync ring is idle during drain: store each
                            # chunk as soon as its copy lands, on
                            # alternating rings.
                            eng = nc.scalar if q == 0 else nc.sync
                            eng.dma_start(
                                out=yt[c * 128:(c + 1) * 128,
                                       tb * TOK:(tb + 1) * TOK],
                                in_=o_t[:, q * TOK:(q + 1) * TOK],
                            )
                    if not last:
                        nc.scalar.dma_start(
                            out=yt[c * 128:(c + 1) * 128,
                                   h * (S // 2):(h + 1) * (S // 2)],
                            in_=o_t[:],
                        )
    nc.finalize()
    return nc


def _prep_in_maps(x, W):
    # Diagonal blocks: Wdiag[g][o, i] = W[g*64+o, g*64+i]
    Wr = W.reshape(G, GS, G, GS)
    g = np.arange(G)
    WdT = Wr[g, :, g, :].transpose(0, 2, 1)          # [g, i, o]
    wb = np.zeros((128, NSTRIP, 128), dtype=np.float32)
    for c in range(NSTRIP):
        wb[0:64, c, 0:64] = WdT[2 * c]
        wb[64:128, c, 64:128] = WdT[2 * c + 1]
    wb = np.ascontiguousarray(wb.reshape(128, NSTRIP * 128)).astype(bfloat16)
    return [
        {"xt": np.ascontiguousarray(x[b].T).astype(bfloat16), "wb": wb}
        for b in range(B)
    ]


def run(x, W, trace=False, **kw):
    x = np.asarray(x, dtype=np.float32)
    W = np.asarray(W, dtype=np.float32)
    nc = _build_program()
    in_maps = _prep_in_maps(x, W)
    res = run_bass_kernel_spmd(nc, in_maps, list(range(B)), trace=trace, **kw)
    y = np.empty((B, S, C), dtype=np.float32)
    for b in range(B):
        y[b] = res.results[b]["yt"].T.astype(np.float32)
    return y, res


def kernel(x, W):
    y, _ = run(x, W, trace=False)
    return y


# revision 3
# speedup vs baseline: 1.0513x; 1.0513x over previous
"""Block-diagonal grouped GEMM (GroupLinear) on 8 TRN2 NeuronCores.

Problem: x [8, 2048, 4096] f32, W [4096, 4096] f32 where only the 64
diagonal 64x64 blocks of W are used:
    y[b,s, g*64+o] = sum_i x[b,s, g*64+i] * W[g*64+o, g*64+i]

Strategy:
  - Data-parallel over batch: core b handles x[b] (2048 tokens).
  - All device I/O in bf16 (rel-err budget 2e-2; bf16 costs ~3e-3),
    halving HBM traffic vs f32: 16MB x + 16MB y + 1MB W per core
    against the ~358 GB/s per-core HBM limit -> ~92us floor.
  - Host pre-packs x strip-major: xs[p, c*2048+s] = x[b][s, c*128+p],
    so the PE contraction dim is on partitions AND any number of
    128-channel strips is one contiguous-per-partition DMA.
  - Two 64-ch groups pack into one 128-wide block-diagonal weight
    strip [128i, 128o]; 32 strips resident in SBUF.
  - Per 2-strip chunk: one 1MB load (sync ring), per strip 4 matmuls
    into a 4-bank PSUM tile, one [128,2048] PSUM->SBUF copy+cast
    (alternating Vector/Scalar: ~450ns fixed cost amortized over 4
    banks), then one 1MB store (scalar ring).  Big DMAs keep the
    ~700ns/trigger HWDGE descriptor-gen cost off the critical path.
  - Host un-permutes ys and upcasts to f32.
"""

import numpy as np
from ml_dtypes import bfloat16

import concourse.bacc as bacc
import concourse.mybir as mybir
from concourse.tile import TileContext
from concourse.bass_utils import run_bass_kernel_spmd

B, S, C = 8, 2048, 4096
G, GS = 64, 64            # groups, group size (=in_scale=out_scale)
NSTRIP = C // 128         # 32 strips of 128 channels (2 groups each)
CHUNK = 2                 # strips per load/store DMA
NCHUNK = NSTRIP // CHUNK
TOK = 512                 # PSUM bank limit: 512 f32 per partition
FP32 = mybir.dt.float32
BF16 = mybir.dt.bfloat16


def _build_program():
    nc = bacc.Bacc()
    xs = nc.declare_dram_parameter("xs", [128, NSTRIP * S], BF16, isOutput=False)
    wb = nc.declare_dram_parameter("wb", [128, NSTRIP * 128], BF16, isOutput=False)
    ys = nc.declare_dram_parameter("ys", [128, NSTRIP * S], BF16, isOutput=True)

    with TileContext(nc) as tc:
        with (
            tc.tile_pool(name="wpool", bufs=1) as wpool,
            tc.tile_pool(name="xpool", bufs=4) as xpool,
            tc.tile_pool(name="opool", bufs=4) as opool,
            tc.tile_pool(name="ppool", bufs=2, space="PSUM") as ppool,
        ):
            # Weight load split across both HWDGE rings so all strips'
            # weights are resident ~2x sooner.
            w_sb = wpool.tile([128, NSTRIP * 128], BF16)
            half = NSTRIP * 128 // 2
            nc.sync.dma_start(out=w_sb[:, :half], in_=wb[:, :half])
            nc.scalar.dma_start(out=w_sb[:, half:], in_=wb[:, half:])
            for cc in range(NCHUNK):
                # Loads issue on the Sync HWDGE ring; stores on the
                # Scalar ring.  A single ring would stall loads behind
                # stores that wait on compute semaphores (FIFO per
                # issuing engine).
                x_t = xpool.tile([128, CHUNK * S], BF16)
                lo = cc * CHUNK * S
                if cc == 0:
                    # Scalar ring is idle during pipeline fill: split
                    # the first chunk across both rings so compute
                    # starts sooner.
                    nc.sync.dma_start(out=x_t[:, :S], in_=xs[:, :S])
                    nc.scalar.dma_start(out=x_t[:, S:], in_=xs[:, S:CHUNK * S])
                else:
                    nc.sync.dma_start(out=x_t[:], in_=xs[:, lo:lo + CHUNK * S])
                o_t = opool.tile([128, CHUNK * S], BF16)
                for k in range(CHUNK):
                    c = cc * CHUNK + k
                    ps = ppool.tile([128, 4 * TOK], FP32)   # 4 PSUM banks
                    for q in range(4):
                        nc.tensor.matmul(
                            out=ps[:, q * TOK:(q + 1) * TOK],
                            lhsT=w_sb[:, c * 128:(c + 1) * 128],
                            rhs=x_t[:, k * S + q * TOK:k * S + (q + 1) * TOK],
                            start=True,
                            stop=True,
                        )
                    # One 4-bank copy+cast per strip, engines alternate.
                    if c % 2 == 0:
                        nc.vector.tensor_copy(
                            out=o_t[:, k * S:(k + 1) * S], in_=ps[:]
                        )
                    else:
                        nc.scalar.copy(
                            out=o_t[:, k * S:(k + 1) * S], in_=ps[:]
                        )
                if cc == NCHUNK - 1:
                    # Sync ring is idle during drain: split the last
                    # store across both rings.
                    nc.scalar.dma_start(
                        out=ys[:, lo:lo + S], in_=o_t[:, :S]
                    )
                    nc.sync.dma_start(
                        out=ys[:, lo + S:lo + CHUNK * S], in_=o_t[:, S:]
                    )
                else:
                    nc.scalar.dma_start(
                        out=ys[:, lo:lo + CHUNK * S], in_=o_t[:]
                    )
    nc.finalize()
    return nc


def _prep_in_maps(x, W):
    # Diagonal blocks: Wdiag[g][o, i] = W[g*64+o, g*64+i]
    Wr = W.reshape(G, GS, G, GS)
    g = np.arange(G)
    WdT = Wr[g, :, g, :].transpose(0, 2, 1)          # [g, i, o]
    wb = np.zeros((128, NSTRIP, 128), dtype=np.float32)
    for c in range(NSTRIP):
        wb[0:64, c, 0:64] = WdT[2 * c]
        wb[64:128, c, 64:128] = WdT[2 * c + 1]
    wb = np.ascontiguousarray(wb.reshape(128, NSTRIP * 128)).astype(bfloat16)
    maps = []
    for b in range(B):
        # xs[p, c*S + s] = x[b][s, c*128 + p]  (strip-major, channel on
        # partitions): [S, NSTRIP, 128] -> [128, NSTRIP, S]
        xs = (
            x[b]
            .reshape(S, NSTRIP, 128)
            .transpose(2, 1, 0)
            .reshape(128, NSTRIP * S)
        )
        maps.append({"xs": np.ascontiguousarray(xs).astype(bfloat16), "wb": wb})
    return maps


def run(x, W, trace=False, **kw):
    x = np.asarray(x, dtype=np.float32)
    W = np.asarray(W, dtype=np.float32)
    nc = _build_program()
    in_maps = _prep_in_maps(x, W)
    res = run_bass_kernel_spmd(nc, in_maps, list(range(B)), trace=trace, **kw)
    y = np.empty((B, S, C), dtype=np.float32)
    for b in range(B):
        ys = res.results[b]["ys"]
        # invert: y[b][s, c*128 + p] = ys[p, c*S + s]
        y[b] = (
            ys.reshape(128, NSTRIP, S)
            .transpose(2, 1, 0)
            .reshape(S, C)
            .astype(np.float32)
        )
    return y, res


def kernel(x, W):
    y, _ = run(x, W, trace=False)
    return y


# revision 6
# speedup vs baseline: 1.0681x; 1.0160x over previous
"""Block-diagonal grouped GEMM (GroupLinear) on 8 TRN2 NeuronCores.

Problem: x [8, 2048, 4096] f32, W [4096, 4096] f32 where only the 64
diagonal 64x64 blocks of W are used:
    y[b,s, g*64+o] = sum_i x[b,s, g*64+i] * W[g*64+o, g*64+i]

Strategy:
  - Data-parallel over batch: core b handles x[b] (2048 tokens).
  - All device I/O in bf16 (rel-err budget 2e-2; bf16 costs ~3e-3),
    halving HBM traffic vs f32: 16MB x + 16MB y + 1MB W per core
    against the ~358 GB/s per-core HBM limit -> ~92us floor.
  - Host pre-packs x strip-major: xs[p, c*2048+s] = x[b][s, c*128+p],
    so the PE contraction dim is on partitions AND any number of
    128-channel strips is one contiguous-per-partition DMA.
  - Two 64-ch groups pack into one 128-wide block-diagonal weight
    strip [128i, 128o]; 32 strips resident in SBUF.
  - Per 2-strip chunk: one 1MB load (sync ring), per strip 4 matmuls
    into a 4-bank PSUM tile, one [128,2048] PSUM->SBUF copy+cast
    (alternating Vector/Scalar: ~450ns fixed cost amortized over 4
    banks), then one 1MB store (scalar ring).  Big DMAs keep the
    ~700ns/trigger HWDGE descriptor-gen cost off the critical path.
  - Host un-permutes ys and upcasts to f32.
"""

import numpy as np
from ml_dtypes import bfloat16

import concourse.bacc as bacc
import concourse.mybir as mybir
from concourse.tile import TileContext
from concourse.bass_utils import run_bass_kernel_spmd

B, S, C = 8, 2048, 4096
G, GS = 64, 64            # groups, group size (=in_scale=out_scale)
NSTRIP = C // 128         # 32 strips of 128 channels (2 groups each)
CHUNK = 4                 # strips per load/store DMA (2MB: ~97% DMA eff.)
NCHUNK = NSTRIP // CHUNK
TOK = 512                 # PSUM bank limit: 512 f32 per partition
FP32 = mybir.dt.float32
BF16 = mybir.dt.bfloat16


def _build_program():
    nc = bacc.Bacc()
    xs = nc.declare_dram_parameter("xs", [128, NSTRIP * S], BF16, isOutput=False)
    wb = nc.declare_dram_parameter("wb", [128, NSTRIP * 128], BF16, isOutput=False)
    ys = nc.declare_dram_parameter("ys", [128, NSTRIP * S], BF16, isOutput=True)

    with TileContext(nc) as tc:
        with (
            tc.tile_pool(name="wpool", bufs=1) as wpool,
            tc.tile_pool(name="xpool", bufs=4) as xpool,
            tc.tile_pool(name="opool", bufs=4) as opool,
            tc.tile_pool(name="ppool", bufs=2, space="PSUM") as ppool,
        ):
            # Weight load split across both HWDGE rings so all strips'
            # weights are resident ~2x sooner.
            w_sb = wpool.tile([128, NSTRIP * 128], BF16)
            half = NSTRIP * 128 // 2
            nc.sync.dma_start(out=w_sb[:, :half], in_=wb[:, :half])
            nc.scalar.dma_start(out=w_sb[:, half:], in_=wb[:, half:])
            for cc in range(NCHUNK):
                # Loads issue on the Sync HWDGE ring; stores on the
                # Scalar ring.  A single ring would stall loads behind
                # stores that wait on compute semaphores (FIFO per
                # issuing engine).
                x_t = xpool.tile([128, CHUNK * S], BF16)
                lo = cc * CHUNK * S
                if cc == 0:
                    # Pipeline fill: strip-granular loads alternating
                    # rings so the first matmul starts ~0.5MB in.
                    for k in range(CHUNK):
                        eng = nc.sync if k % 2 == 0 else nc.scalar
                        eng.dma_start(
                            out=x_t[:, k * S:(k + 1) * S],
                            in_=xs[:, k * S:(k + 1) * S],
                        )
                else:
                    nc.sync.dma_start(out=x_t[:], in_=xs[:, lo:lo + CHUNK * S])
                o_t = opool.tile([128, CHUNK * S], BF16)
                for k in range(CHUNK):
                    c = cc * CHUNK + k
                    ps = ppool.tile([128, 4 * TOK], FP32)   # 4 PSUM banks
                    for q in range(4):
                        nc.tensor.matmul(
                            out=ps[:, q * TOK:(q + 1) * TOK],
                            lhsT=w_sb[:, c * 128:(c + 1) * 128],
                            rhs=x_t[:, k * S + q * TOK:k * S + (q + 1) * TOK],
                            start=True,
                            stop=True,
                        )
                    # One 4-bank copy+cast per strip, engines alternate.
                    if c % 2 == 0:
                        nc.vector.tensor_copy(
                            out=o_t[:, k * S:(k + 1) * S], in_=ps[:]
                        )
                    else:
                        nc.scalar.copy(
                            out=o_t[:, k * S:(k + 1) * S], in_=ps[:]
                        )
                if cc == NCHUNK - 1:
                    # Drain: strip-granular stores alternating rings so
                    # the final transfer tail is ~0.5MB, not 2MB.
                    for k in range(CHUNK):
                        eng = nc.scalar if k % 2 == 0 else nc.sync
                        eng.dma_start(
                            out=ys[:, lo + k * S:lo + (k + 1) * S],
                            in_=o_t[:, k * S:(k + 1) * S],
                        )
                else:
                    nc.scalar.dma_start(
                        out=ys[:, lo:lo + CHUNK * S], in_=o_t[:]
                    )
    nc.finalize()
    return nc


def _prep_in_maps(x, W):
    # Diagonal blocks: Wdiag[g][o, i] = W[g*64+o, g*64+i]
    Wr = W.reshape(G, GS, G, GS)
    g = np.arange(G)
    WdT = Wr[g, :, g, :].transpose(0, 2, 1)          # [g, i, o]
    wb = np.zeros((128, NSTRIP, 128), dtype=np.float32)
    for c in range(NSTRIP):
        wb[0:64, c, 0:64] = WdT[2 * c]
        wb[64:128, c, 64:128] = WdT[2 * c + 1]
    wb = np.ascontiguousarray(wb.reshape(128, NSTRIP * 128)).astype(bfloat16)
    maps = []
    for b in range(B):
        # xs[p, c*S + s] = x[b][s, c*128 + p]  (strip-major, channel on
        # partitions): [S, NSTRIP, 128] -> [128, NSTRIP, S]
        xs = (
            x[b]
            .reshape(S, NSTRIP, 128)
            .transpose(2, 1, 0)
            .reshape(128, NSTRIP * S)
        )
        maps.append({"xs": np.ascontiguousarray(xs).astype(bfloat16), "wb": wb})
    return maps


def run(x, W, trace=False, **kw):
    x = np.asarray(x, dtype=np.float32)
    W = np.asarray(W, dtype=np.float32)
    nc = _build_program()
    in_maps = _prep_in_maps(x, W)
    res = run_bass_kernel_spmd(nc, in_maps, list(range(B)), trace=trace, **kw)
    y = np.empty((B, S, C), dtype=np.float32)
    for b in range(B):
        ys = res.results[b]["ys"]
        # invert: y[b][s, c*128 + p] = ys[p, c*S + s]
        y[b] = (
            ys.reshape(128, NSTRIP, S)
            .transpose(2, 1, 0)
            .reshape(S, C)
            .astype(np.float32)
        )
    return y, res


def kernel(x, W):
    y, _ = run(x, W, trace=False)
    return y
